# revision 37
# baseline (speedup 1.0000x reference)
"""Local (windowed) attention with shared KV head — TRN2 Bass kernel, v4.

Problem: b=1, L=4096, d_model=1024, n_head=16, d_head=64, w=512.
  qp = (q@Wq)/8; k,v = kv@Wkv; per 512-chunk attention over {prev,self,next}
  chunks with zero-padded edges (softmax includes exp(0)=1 terms for pads);
  out = ctx @ Wo.

Sharding: sequence-parallel over the 8 chunks, one chunk per NeuronCore.
Each core recomputes the K/V projection for its 3-chunk halo (no collectives).

Design notes (evolved from the 381us baseline):
  - ScalarE exp is the roofline (~12.6M elements/core ~ 82+ us at 1
    elem/lane/cyc @1.2GHz + ~300cyc/instr overhead). The loop streams exp
    continuously: per (head-pair, y-block) one [128,1024] PSUM score tile
    (A|B halves via row-tiled concurrent K=64 matmuls at tile_position
    (0,0)/(64,0)) -> ONE wide ACTIVATE -> bf16 P tile -> two ctx matmuls
    (v65: 65th lhsT row of ones accumulates softmax denominator Z for free).
  - Scores double-buffered (2x2 PSUM banks) so ACT never waits on the PE.
  - bf16 datapath: halves DMA bytes/SBUF (PE rate unchanged, err ~5e-3).
  - reciprocal_approx_fast (base-partition-0 only!) on a packed [65,1024]
    tile replaces 4us-per-head iterative reciprocal.
  - Each dma_start costs ~600ns of *serialized submission* on its issuing
    engine's queue; submissions are spread across vector/gpsimd/scalar/sync
    so the ramp isn't gated by one queue.
  - q-projection for pair i+2 interleaves into pair i's groups; pair i-1's
    normalization matmuls defer into pair i's early groups (no PE
    head-of-line blocking of the ACT stream at pair boundaries).
  - out-projection is a dense tail; OUT is stored bf16 and widened on host.

PSUM banks in steady state: scores 2x[128,1024] (4) + cxA/cxB (2) + qp (1)
  + zbc (1) = 8.
"""

import numpy as np

B, L, DM, NH, DH, W = 1, 4096, 1024, 16, 64, 512
NCORES = 8
CH = L // NCORES        # 512 tokens per core
YW = 3 * W              # 1536 halo positions
P = 128
NF = DM // P            # 8 feature tiles
NY = YW // P            # 12 y blocks
NPAIR = NH // 2         # 8 head pairs

_CACHE = {}


def _build():
    import concourse.mybir as mybir
    import concourse.tile as tile
    from concourse import bacc
    from concourse.masks import make_identity
    from contextlib import ExitStack

    F32 = mybir.dt.float32
    BF16 = mybir.dt.bfloat16
    FP8 = mybir.dt.float8e4
    EXP = mybir.ActivationFunctionType.Exp

    nc = bacc.Bacc("TRN2", target_bir_lowering=False, debug=False)
    # Host pre-tiles every input so each is ONE contiguous 2D DMA
    # (a dma_start costs ~650ns serialized on its queue; granularity kills).
    QT = nc.dram_tensor("QT", [P, NF * CH], BF16, kind="ExternalInput")
    KVT = nc.dram_tensor("KVT", [P, 3 * NF * W], BF16, kind="ExternalInput")
    WQ = nc.dram_tensor("WQ", [P, 2 * NF * (DM // 2)], BF16, kind="ExternalInput")
    WVK = nc.dram_tensor("WVK", [P, NF * P], BF16, kind="ExternalInput")  # [Wv | Wk]
    WO = nc.dram_tensor("WO", [P, NF * DM], BF16, kind="ExternalInput")
    OUT = nc.dram_tensor("OUT", [CH, DM], BF16, kind="ExternalOutput")

    with tile.TileContext(nc) as tc, ExitStack() as ctx, \
         nc.allow_low_precision(reason="bf16 datapath; rel-err budget 2e-2"):
        perm = ctx.enter_context(tc.tile_pool(name="perm", bufs=1))

        identb = perm.tile([64, 64], BF16, tag="identb")
        make_identity(nc, identb[:])
        identF = perm.tile([P, P], BF16, tag="identF")
        make_identity(nc, identF[:])
        # ones row for the 1/Z broadcast matmuls (row 64 to stay lane-aligned
        # with the Z row of the ctx PSUM tiles)
        onesEb = perm.tile([65, 64], BF16, tag="onesEb")
        nc.vector.memset(onesEb[64:65, :], 1.0)

        # --- persistent SBUF tiles (bf16)
        wvkS = perm.tile([P, NF * P], BF16, tag="wvkS")

        def wvk(f):
            return wvkS[:, P * f:P * (f + 1)]
        wqS = perm.tile([P, 2 * NF * (DM // 2)], BF16, tag="wqS")
        woS = perm.tile([P, NF * DM], BF16, tag="woS")
        qtS = perm.tile([P, NF * CH], BF16, tag="qtS")

        def qt(f):
            return qtS[:, CH * f:CH * (f + 1)]

        def wqm(f, m):
            h, mm = (0, m) if m < 4 else (1, m - 4)
            base = 4096 * h + 512 * f + 128 * mm
            return wqS[:, base:base + 128]

        def wo(j):
            return woS[:, DM * j:DM * (j + 1)]
        k3T2 = perm.tile([P, YW], BF16, tag="k3T2")
        vTs = perm.tile([64, YW], BF16, tag="vTs")
        v65 = [perm.tile([P, 65], BF16, tag=f"v65_{t}", name=f"v65_{t}") for t in range(NY)]
        qpT = [perm.tile([P, CH], BF16, tag=f"qpT{m}", name=f"qpT{m}") for m in range(NF)]
        ctxn = [perm.tile([P, CH], BF16, tag=f"ctxn{i}", name=f"ctxn{i}") for i in range(NPAIR)]
        oacc = [perm.tile([P, W], BF16, tag=f"oacc{t}", name=f"oacc{t}") for t in range(8)]

        # SBUF placement pad: restores the P-pool address at which ACT exp
        # measured 1113ns instead of 1335ns (layout-sensitive overhead).
        sbpad = perm.tile([P, 6656], BF16, tag="sbpad")

        qpp = ctx.enter_context(tc.tile_pool(name="qpps", bufs=1, space="PSUM"))

        # ---------------- ramp (NO scoped pools: a pool close acts as a
        # barrier that would gate the first exp on all of chunk-1/2 work)
        kvtS = [perm.tile([P, NF * W], BF16, tag=f"kvtS{n}", name=f"kvtS{n}")
                for n in range(3)]

        def kvt(n, f):
            return kvtS[n][:, W * f:W * (f + 1)]

        # Input DMAs on the sync queue in strict priority order; kv chunks
        # 1/2 in quarters so their projections can start on partial data.
        NW = NF * W
        nc.sync.dma_start(wvkS[:], WVK.ap()[:, :])
        nc.sync.dma_start(kvtS[0][:], KVT.ap()[:, 0:NW])
        nc.sync.dma_start(qtS[:], QT.ap()[:, :])
        nc.sync.dma_start(wqS[:, 0:4096], WQ.ap()[:, 0:4096])
        for n in range(1, 3):
            for h in range(4):
                nc.sync.dma_start(kvtS[n][:, 1024 * h:1024 * (h + 1)],
                                  KVT.ap()[:, n * NW + 1024 * h:n * NW + 1024 * (h + 1)])
        nc.sync.dma_start(wqS[:, 4096:8192], WQ.ap()[:, 4096:8192])
        nc.sync.dma_start(woS[:], WO.ap()[:, :])

        def do_chunk(n):
            # kv projection: [128,512] psum; rows 0:64=vT, 64:128=kT.
            # Shares the persistent qp PSUM tag (no scoped pool).
            ns = slice(W * n, W * (n + 1))
            ps = qpp.tile([P, CH], F32, tag="qp", name=f"kvp{n}")
            for f in range(NF):
                nc.tensor.matmul(ps[:], wvk(f), kvt(n, f),
                                 start=(f == 0), stop=(f == NF - 1))
            nc.vector.tensor_copy(k3T2[64:128, ns], ps[64:128, :])
            nc.vector.tensor_copy(vTs[:, ns], ps[0:64, :])
            # kT partition dup + vT transposes via gpsimd DMA (xbar), keeping
            # the PE and the sync input queue out of the way
            nc.gpsimd.dma_start(k3T2[0:64, ns], k3T2[64:128, ns])
            # chunk 0 transposes ride the idle scalar queue (pre-exp); later
            # chunks go on sync after the input transfers have drained
            xeng = nc.scalar if n == 0 else nc.sync
            for t in range(4 * n, 4 * n + 4):
                xeng.dma_start_transpose(v65[t][:, 0:64],
                                         vTs[:, P * t:P * (t + 1)])
        for t in range(NY):
            nc.vector.memset(v65[t][:, 64:65], 1.0)

        do_chunk(0)
        # q projection for pairs 0 and 1 (rest interleaved into the loop)
        for m in range(2):
            ps = qpp.tile([P, CH], F32, tag="qp", name="qp")
            for f in range(NF):
                nc.tensor.matmul(ps[:], wqm(f, m), qt(f),
                                 start=(f == 0), stop=(f == NF - 1))
            nc.vector.tensor_copy(qpT[m][:], ps[:])

        # ---------------- attention main loop
        with tc.tile_pool(name="scps", bufs=2, space="PSUM") as scp, \
             tc.tile_pool(name="cxps", bufs=1, space="PSUM") as cxp, \
             tc.tile_pool(name="zbps", bufs=1, space="PSUM") as zbp, \
             tc.tile_pool(name="pt", bufs=4) as ptp, \
             tc.tile_pool(name="nrm", bufs=2) as nrm:

            pending_norm = [None]   # deferred zbc+mul emission
            pending_prologue = [None]

            def out_slices(t):
                x, o = t // 2, t % 2
                return slice(P * x, P * (x + 1)), slice(W * o, W * (o + 1))

            def emit_out_partial(t, jhi):
                # partial out-projection over pairs 0..jhi -> SBUF accumulator
                xs, os_ = out_slices(t)
                ps = qpp.tile([P, CH], F32, tag="qp", name=f"opp{t}")
                for j in range(jhi + 1):
                    nc.tensor.matmul(ps[:], ctxn[j][:, xs], wo(j)[:, os_],
                                     start=(j == 0), stop=(j == jhi))
                nc.vector.tensor_copy(oacc[t][:], ps[:])

            def emit_out_mid(t, jlo, jhi):
                # pairs jlo..jhi + previous partial, accumulated on the PE via
                # an identity matmul (ps += I.T @ oacc) -- no DVE adds
                xs, os_ = out_slices(t)
                ps = qpp.tile([P, CH], F32, tag="qp", name=f"opj6{t}")
                nc.tensor.matmul(ps[:], identF[:], oacc[t][:],
                                 start=True, stop=False)
                for j in range(jlo, jhi + 1):
                    nc.tensor.matmul(ps[:], ctxn[j][:, xs], wo(j)[:, os_],
                                     start=False, stop=(j == jhi))
                nc.vector.tensor_copy(oacc[t][:], ps[:])

            # (pair, group) -> (tile, jlo, jhi): jhi <= pair-2 before g4
            # (norm of pair-1 lands at g4), <= pair-1 from g5 on; in pairs 3-5
            # the qp psum bank is busy with qproj between g3 and g11.
            OUT_SCHED = {
                (3, 1): (0, 0, 1), (3, 11): (1, 0, 2),
                (4, 1): (2, 0, 2), (4, 11): (3, 0, 3),
                (5, 1): (4, 0, 3), (5, 11): (5, 0, 4),
                (6, 5): (6, 0, 5), (6, 7): (7, 0, 5),
                (6, 9): (0, 2, 5), (6, 11): (1, 3, 5),
                (7, 5): (2, 3, 6), (7, 6): (3, 4, 6),
                (7, 7): (4, 4, 6), (7, 8): (5, 5, 6),
            }
            TAIL_JLO = {0: 6, 1: 6, 2: 7, 3: 7, 4: 7, 5: 7, 6: 6, 7: 6}

            # Flat software pipeline: scores/exp at step s, ctx lags 2 steps so
            # a pair's last ctx matmuls never head-of-line-block the next
            # pair's scores (the ACT stream stays dense at pair boundaries).
            LAG = 2
            cxs_ = {}
            pgs = {}
            qp_ps = [None]

            def emit_ctx(i, g):
                if g == 0:
                    cxs_[i] = (cxp.tile([65, W], F32, tag="cxA", name="cxA"),
                               cxp.tile([65, W], F32, tag="cxB", name="cxB"))
                cxA, cxB = cxs_[i]
                st, sp = (g == 0), (g == NY - 1)
                nc.tensor.matmul(cxA[:, :], v65[g][:], pgs[(i, g)][:, 0:W],
                                 start=st, stop=sp)
                nc.tensor.matmul(cxB[:, :], v65[g][:], pgs[(i, g)][:, W:2 * W],
                                 start=st, stop=sp)
                del pgs[(i, g)]

            def emit_prologue(i):
                cxA, cxB = cxs_[i]
                Zp = nrm.tile([65, 2 * W], F32, tag="Zp", name="Zp")
                zinv = nrm.tile([65, 2 * W], F32, tag="zinv", name="zinv")
                zinvb = nrm.tile([65, 2 * W], BF16, tag="zinvb", name="zinvb")
                nc.vector.tensor_copy(Zp[64:65, 0:W], cxA[64:65, :])
                nc.vector.tensor_copy(Zp[64:65, W:2 * W], cxB[64:65, :])
                # evacuate ctx first: releases the cx PSUM banks for the next
                # pair's ctx accumulation before the slow reciprocal runs
                cxsA = nrm.tile([64, W], BF16, tag="cxsA", name="cxsA")
                cxsB = nrm.tile([64, W], BF16, tag="cxsB", name="cxsB")
                nc.vector.tensor_copy(cxsA[:], cxA[0:64, :])
                nc.vector.tensor_copy(cxsB[:], cxB[0:64, :])
                # custom DVE op requires base partition 0: run over all 65
                # rows (row 64 holds Z_A|Z_B, the rest are don't-care lanes)
                nc.vector.reciprocal_approx_fast(zinv[:], Zp[:])
                nc.vector.tensor_copy(zinvb[64:65, :], zinv[64:65, :])
                del cxs_[i]

                def norm_tail():
                    zbA = zbp.tile([64, W], F32, tag="zb", name="zbA")
                    nc.tensor.matmul(zbA[:], onesEb[64:65, :], zinvb[64:65, 0:W],
                                     start=True, stop=True, tile_position=(64, 0))
                    nc.vector.tensor_mul(ctxn[i][0:64, :], cxsA[:], zbA[:])
                    zbB = zbp.tile([64, W], F32, tag="zb", name="zbB")
                    nc.tensor.matmul(zbB[:], onesEb[64:65, :],
                                     zinvb[64:65, W:2 * W],
                                     start=True, stop=True, tile_position=(64, 0))
                    cbt = nrm.tile([64, W], BF16, tag="cbt", name="cbt")
                    nc.vector.tensor_mul(cbt[:], cxsB[:], zbB[:])
                    nc.sync.dma_start(ctxn[i][64:128, :], cbt[:])
                return norm_tail

            for s in range(NPAIR * NY + LAG):
                i, g = divmod(s, NY)
                if s < NPAIR * NY:
                    # scores + exp for (i, g)
                    ys = slice(P * g, P * (g + 1))
                    scS = scp.tile([P, 2 * W], F32, tag="sc", name="sc")
                    nc.tensor.matmul(scS[:, 0:W], k3T2[0:64, ys],
                                     qpT[i][0:64, :], start=True, stop=True,
                                     tile_position=(0, 0))
                    nc.tensor.matmul(scS[:, W:2 * W], k3T2[64:128, ys],
                                     qpT[i][64:128, :], start=True, stop=True,
                                     tile_position=(64, 0))
                    pt_ = ptp.tile([P, 2 * W], BF16, tag="pt", name="pt")
                    nc.scalar.activation(pt_[:], scS[:], EXP)
                    pgs[(i, g)] = pt_
                if s >= LAG:
                    ci, cg = divmod(s - LAG, NY)
                    emit_ctx(ci, cg)
                    if cg == NY - 1:
                        pending_prologue[0] = ci
                if s < NPAIR * NY:
                    if g == 2 and pending_prologue[0] is not None:
                        pending_norm[0] = emit_prologue(pending_prologue[0])
                        pending_prologue[0] = None
                    if g == 5 and pending_norm[0] is not None:
                        pending_norm[0]()
                        pending_norm[0] = None
                    if i == 0 and g == 3:
                        # de-prioritized: the kv c1/c2 DMAs land late; the
                        # scheduler must prefer ready scores over these MMs
                        with tc.high_priority(offset=-100000):
                            do_chunk(1)
                    if i == 0 and g == 6:
                        with tc.high_priority(offset=-100000):
                            do_chunk(2)
                    # q projection for pair i+2, one f-tile per group
                    m = i + 2
                    if m < NPAIR and 3 <= g <= 10:
                        f = g - 3
                        if f == 0:
                            qp_ps[0] = qpp.tile([P, CH], F32, tag="qp", name="qp2")
                        nc.tensor.matmul(qp_ps[0][:], wqm(f, m),
                                         qt(f), start=(f == 0), stop=(f == NF - 1))
                    if m < NPAIR and g == 11:
                        nc.vector.tensor_copy(qpT[m][:], qp_ps[0][:])
                    # out-projection partials ride the idle qp bank in pairs
                    # 6-7; jhi respects which ctxn normalizations are emitted
                    # (pair i-1's lands at g5 of pair i)
                    ent = {(6, 1): (0, 4), (6, 3): (1, 4),
                           (6, 5): (2, 5), (6, 7): (3, 5),
                           (7, 1): (4, 5), (7, 3): (5, 5),
                           (7, 5): (6, 6), (7, 7): (7, 6)}.get((i, g))
                    if ent is not None:
                        t, jhi = ent
                        xs, os_ = (slice(P * (t // 2), P * (t // 2 + 1)),
                                   slice(W * (t % 2), W * (t % 2 + 1)))
                        pso = qpp.tile([P, CH], F32, tag="qp", name=f"op{t}")
                        for j in range(jhi + 1):
                            nc.tensor.matmul(pso[:], ctxn[j][:, xs],
                                             wo(j)[:, os_],
                                             start=(j == 0), stop=(j == jhi))
                        nc.vector.tensor_copy(oacc[t][:], pso[:])
            # pair 7 epilogue (must run inside this scope: zbp/nrm pools)
            emit_prologue(pending_prologue[0])()

        # ---------------- output projection tail
        with tc.tile_pool(name="opps", bufs=4, space="PSUM") as opp, \
             tc.tile_pool(name="osb", bufs=4) as osb:
            TAIL_JLO = {0: 5, 1: 5, 2: 6, 3: 6, 4: 6, 5: 6, 6: 7, 7: 7}
            for t in range(8):
                x, o = t // 2, t % 2
                xs = slice(P * x, P * (x + 1))
                os_ = slice(W * o, W * (o + 1))
                ps = opp.tile([P, W], F32, tag="op", name="op")
                nc.tensor.matmul(ps[:], identF[:], oacc[t][:],
                                 start=True, stop=False)
                for j in range(TAIL_JLO[t], NPAIR):
                    nc.tensor.matmul(ps[:], ctxn[j][:, xs], wo(j)[:, os_],
                                     start=False, stop=(j == NPAIR - 1))
                ot = osb.tile([P, W], BF16, tag="os", name="os")
                nc.scalar.copy(ot[:], ps[:])
                nc.sync.dma_start(OUT.ap()[xs, os_], ot[:])

    nc.compile()
    return nc


def _get_nc():
    if "nc" not in _CACHE:
        _CACHE["nc"] = _build()
    return _CACHE["nc"]


def kernel(q, kv, Wq, Wkv, Wo, w=None, _trace=False):
    import ml_dtypes
    from concourse import bass_utils

    BF = ml_dtypes.bfloat16
    F8 = ml_dtypes.float8_e4m3

    q = np.asarray(q, np.float32).reshape(L, DM)
    kv = np.asarray(kv, np.float32).reshape(L, DM)
    Wq = np.asarray(Wq, np.float32)
    Wkv = np.asarray(Wkv, np.float32)
    Wo = np.asarray(Wo, np.float32)

    def tile_rows(a, dt):
        # [DM, C] -> [128, NF*C] with column block f = rows 128f:128(f+1)
        c = a.shape[1]
        out = np.empty((P, a.shape[0] // P * c), dt)
        for f in range(a.shape[0] // P):
            out[:, f * c:(f + 1) * c] = a[P * f:P * (f + 1), :]
        return np.ascontiguousarray(out)

    qT = q.T.astype(np.float32)                      # [DM, L]
    kvT = kv.T.astype(np.float32)                    # [DM, L]
    WQs = (Wq / np.sqrt(DH)).astype(np.float32)      # fold 1/sqrt(d_head)
    WVKf = np.concatenate([Wkv[:, DH:], Wkv[:, :DH]], axis=1)  # [Wv | Wk]

    # wq layout: [128, 2*NF*512]: block (h, f) = WQs[128f:128(f+1), 512h:512(h+1)]
    wq_t = np.empty((P, 2 * NF * (DM // 2)), BF)
    for h in range(2):
        for f in range(NF):
            wq_t[:, 4096 * h + 512 * f: 4096 * h + 512 * (f + 1)] = \
                WQs[P * f:P * (f + 1), 512 * h:512 * (h + 1)].astype(BF)
    wq_t = np.ascontiguousarray(wq_t)
    wvk_t = tile_rows(WVKf, BF)
    wo_t = tile_rows(Wo, BF)

    in_maps = []
    for c in range(NCORES):
        kvt_c = np.zeros((DM, YW), np.float32)
        lo = (c - 1) * CH
        hi = (c + 2) * CH
        src_lo, src_hi = max(lo, 0), min(hi, L)
        dst_lo = src_lo - lo
        kvt_c[:, dst_lo:dst_lo + (src_hi - src_lo)] = kvT[:, src_lo:src_hi]
        # kv layout: [128, 3*NF*512]: block (n, f) = chunk n, rows 128f:
        kv_t = np.empty((P, 3 * NF * W), BF)
        for n in range(3):
            for f in range(NF):
                kv_t[:, (n * NF + f) * W:(n * NF + f + 1) * W] = \
                    kvt_c[P * f:P * (f + 1), W * n:W * (n + 1)].astype(BF)
        in_maps.append({
            "QT": tile_rows(np.ascontiguousarray(qT[:, c * CH:(c + 1) * CH]), BF),
            "KVT": np.ascontiguousarray(kv_t),
            "WQ": wq_t,
            "WVK": wvk_t,
            "WO": wo_t,
        })

    nc = _get_nc()
    res = bass_utils.run_bass_kernel_spmd(
        nc, in_maps, core_ids=list(range(NCORES)), trace=_trace)
    if _trace:
        _CACHE["last_result"] = res

    out = np.concatenate([np.asarray(r["OUT"]).astype(np.float32)
                          for r in res.results], axis=0)
    return out.reshape(B, L, DM)


# revision 38
# speedup vs baseline: 1.0324x; 1.0324x over previous
"""Local (windowed) attention with shared KV head — TRN2 Bass kernel, v4.

Problem: b=1, L=4096, d_model=1024, n_head=16, d_head=64, w=512.
  qp = (q@Wq)/8; k,v = kv@Wkv; per 512-chunk attention over {prev,self,next}
  chunks with zero-padded edges (softmax includes exp(0)=1 terms for pads);
  out = ctx @ Wo.

Sharding: sequence-parallel over the 8 chunks, one chunk per NeuronCore.
Each core recomputes the K/V projection for its 3-chunk halo (no collectives).

Design notes (evolved from the 381us baseline):
  - ScalarE exp is the roofline (~12.6M elements/core ~ 82+ us at 1
    elem/lane/cyc @1.2GHz + ~300cyc/instr overhead). The loop streams exp
    continuously: per (head-pair, y-block) one [128,1024] PSUM score tile
    (A|B halves via row-tiled concurrent K=64 matmuls at tile_position
    (0,0)/(64,0)) -> ONE wide ACTIVATE -> bf16 P tile -> two ctx matmuls
    (v65: 65th lhsT row of ones accumulates softmax denominator Z for free).
  - Scores double-buffered (2x2 PSUM banks) so ACT never waits on the PE.
  - bf16 datapath: halves DMA bytes/SBUF (PE rate unchanged, err ~5e-3).
  - reciprocal_approx_fast (base-partition-0 only!) on a packed [65,1024]
    tile replaces 4us-per-head iterative reciprocal.
  - Each dma_start costs ~600ns of *serialized submission* on its issuing
    engine's queue; submissions are spread across vector/gpsimd/scalar/sync
    so the ramp isn't gated by one queue.
  - q-projection for pair i+2 interleaves into pair i's groups; pair i-1's
    normalization matmuls defer into pair i's early groups (no PE
    head-of-line blocking of the ACT stream at pair boundaries).
  - out-projection is a dense tail; OUT is stored bf16 and widened on host.

PSUM banks in steady state: scores 2x[128,1024] (4) + cxA/cxB (2) + qp (1)
  + zbc (1) = 8.
"""

import numpy as np

B, L, DM, NH, DH, W = 1, 4096, 1024, 16, 64, 512
NCORES = 8
CH = L // NCORES        # 512 tokens per core
YW = 3 * W              # 1536 halo positions
P = 128
NF = DM // P            # 8 feature tiles
NY = YW // P            # 12 y blocks
NPAIR = NH // 2         # 8 head pairs

_CACHE = {}


def _build():
    import concourse.mybir as mybir
    import concourse.tile as tile
    from concourse import bacc
    from concourse.masks import make_identity
    from contextlib import ExitStack

    F32 = mybir.dt.float32
    BF16 = mybir.dt.bfloat16
    FP8 = mybir.dt.float8e4
    EXP = mybir.ActivationFunctionType.Exp

    nc = bacc.Bacc("TRN2", target_bir_lowering=False, debug=False)
    # Host pre-tiles every input so each is ONE contiguous 2D DMA
    # (a dma_start costs ~650ns serialized on its queue; granularity kills).
    QT = nc.dram_tensor("QT", [P, NF * CH], BF16, kind="ExternalInput")
    KVT = nc.dram_tensor("KVT", [P, 3 * NF * W], BF16, kind="ExternalInput")
    WQ = nc.dram_tensor("WQ", [P, 2 * NF * (DM // 2)], BF16, kind="ExternalInput")
    WVK = nc.dram_tensor("WVK", [P, NF * P], BF16, kind="ExternalInput")  # [Wv | Wk]
    WO = nc.dram_tensor("WO", [P, NF * DM], BF16, kind="ExternalInput")
    OUT = nc.dram_tensor("OUT", [CH, DM], BF16, kind="ExternalOutput")

    with tile.TileContext(nc) as tc, ExitStack() as ctx, \
         nc.allow_low_precision(reason="bf16 datapath; rel-err budget 2e-2"):
        perm = ctx.enter_context(tc.tile_pool(name="perm", bufs=1))

        identb = perm.tile([64, 64], BF16, tag="identb")
        make_identity(nc, identb[:])
        identF = perm.tile([P, P], BF16, tag="identF")
        make_identity(nc, identF[:])
        # ones row for the 1/Z broadcast matmuls (row 64 to stay lane-aligned
        # with the Z row of the ctx PSUM tiles)
        onesEb = perm.tile([65, 64], BF16, tag="onesEb")
        nc.vector.memset(onesEb[64:65, :], 1.0)

        # --- persistent SBUF tiles (bf16)
        wvkS = perm.tile([P, NF * P], BF16, tag="wvkS")

        def wvk(f):
            return wvkS[:, P * f:P * (f + 1)]
        wqS = perm.tile([P, 2 * NF * (DM // 2)], BF16, tag="wqS")
        woS = perm.tile([P, NF * DM], BF16, tag="woS")
        qtS = perm.tile([P, NF * CH], BF16, tag="qtS")

        def qt(f):
            return qtS[:, CH * f:CH * (f + 1)]

        def wqm(f, m):
            h, mm = (0, m) if m < 4 else (1, m - 4)
            base = 4096 * h + 512 * f + 128 * mm
            return wqS[:, base:base + 128]

        def wo(j):
            return woS[:, DM * j:DM * (j + 1)]
        k3T2 = perm.tile([P, YW], BF16, tag="k3T2")
        vTs = perm.tile([64, YW], BF16, tag="vTs")
        v65 = [perm.tile([P, 65], BF16, tag=f"v65_{t}", name=f"v65_{t}") for t in range(NY)]
        qpT = [perm.tile([P, CH], BF16, tag=f"qpT{m}", name=f"qpT{m}") for m in range(NF)]
        ctxn = [perm.tile([P, CH], BF16, tag=f"ctxn{i}", name=f"ctxn{i}") for i in range(NPAIR)]
        oacc = [perm.tile([P, W], BF16, tag=f"oacc{t}", name=f"oacc{t}") for t in range(8)]

        # SBUF placement pad: restores the P-pool address at which ACT exp
        # measured 1113ns instead of 1335ns (layout-sensitive overhead).
        sbpad = perm.tile([P, 6656], BF16, tag="sbpad")

        qpp = ctx.enter_context(tc.tile_pool(name="qpps", bufs=1, space="PSUM"))

        # ---------------- ramp (NO scoped pools: a pool close acts as a
        # barrier that would gate the first exp on all of chunk-1/2 work)
        kvtS = [perm.tile([P, NF * W], BF16, tag=f"kvtS{n}", name=f"kvtS{n}")
                for n in range(3)]

        def kvt(n, f):
            return kvtS[n][:, W * f:W * (f + 1)]

        # Input DMAs on the sync queue in strict priority order; kv chunks
        # 1/2 in quarters so their projections can start on partial data.
        NW = NF * W
        nc.sync.dma_start(wvkS[:], WVK.ap()[:, :])
        nc.sync.dma_start(kvtS[0][:], KVT.ap()[:, 0:NW])
        nc.sync.dma_start(qtS[:], QT.ap()[:, :])
        nc.sync.dma_start(wqS[:, 0:4096], WQ.ap()[:, 0:4096])
        for n in range(1, 3):
            for h in range(4):
                nc.sync.dma_start(kvtS[n][:, 1024 * h:1024 * (h + 1)],
                                  KVT.ap()[:, n * NW + 1024 * h:n * NW + 1024 * (h + 1)])
        nc.sync.dma_start(wqS[:, 4096:8192], WQ.ap()[:, 4096:8192])
        nc.sync.dma_start(woS[:], WO.ap()[:, :])

        def do_chunk(n):
            # kv projection: [128,512] psum; rows 0:64=vT, 64:128=kT.
            # Shares the persistent qp PSUM tag (no scoped pool).
            ns = slice(W * n, W * (n + 1))
            ps = qpp.tile([P, CH], F32, tag="qp", name=f"kvp{n}")
            for f in range(NF):
                nc.tensor.matmul(ps[:], wvk(f), kvt(n, f),
                                 start=(f == 0), stop=(f == NF - 1))
            nc.vector.tensor_copy(k3T2[64:128, ns], ps[64:128, :])
            nc.vector.tensor_copy(vTs[:, ns], ps[0:64, :])
            # kT partition dup + vT transposes via gpsimd DMA (xbar), keeping
            # the PE and the sync input queue out of the way
            nc.gpsimd.dma_start(k3T2[0:64, ns], k3T2[64:128, ns])
            # chunk 0 transposes ride the idle scalar queue (pre-exp); later
            # chunks go on sync after the input transfers have drained
            xeng = nc.scalar if n == 0 else nc.sync
            for t in range(4 * n, 4 * n + 4):
                xeng.dma_start_transpose(v65[t][:, 0:64],
                                         vTs[:, P * t:P * (t + 1)])
        for t in range(NY):
            nc.vector.memset(v65[t][:, 64:65], 1.0)

        do_chunk(0)
        # q projection for pairs 0 and 1 (rest interleaved into the loop)
        for m in range(2):
            ps = qpp.tile([P, CH], F32, tag="qp", name="qp")
            for f in range(NF):
                nc.tensor.matmul(ps[:], wqm(f, m), qt(f),
                                 start=(f == 0), stop=(f == NF - 1))
            nc.vector.tensor_copy(qpT[m][:], ps[:])

        # ---------------- attention main loop
        with tc.tile_pool(name="scps", bufs=2, space="PSUM") as scp, \
             tc.tile_pool(name="cxps", bufs=1, space="PSUM") as cxp, \
             tc.tile_pool(name="zbps", bufs=1, space="PSUM") as zbp, \
             tc.tile_pool(name="pt", bufs=4) as ptp, \
             tc.tile_pool(name="nrm", bufs=2) as nrm:

            pending_norm = [None]   # deferred zbc+mul emission
            pending_prologue = [None]

            def out_slices(t):
                x, o = t // 2, t % 2
                return slice(P * x, P * (x + 1)), slice(W * o, W * (o + 1))

            def emit_out_partial(t, jhi):
                # partial out-projection over pairs 0..jhi -> SBUF accumulator
                xs, os_ = out_slices(t)
                ps = qpp.tile([P, CH], F32, tag="qp", name=f"opp{t}")
                for j in range(jhi + 1):
                    nc.tensor.matmul(ps[:], ctxn[j][:, xs], wo(j)[:, os_],
                                     start=(j == 0), stop=(j == jhi))
                nc.vector.tensor_copy(oacc[t][:], ps[:])

            def emit_out_mid(t, jlo, jhi):
                # pairs jlo..jhi + previous partial, accumulated on the PE via
                # an identity matmul (ps += I.T @ oacc) -- no DVE adds
                xs, os_ = out_slices(t)
                ps = qpp.tile([P, CH], F32, tag="qp", name=f"opj6{t}")
                nc.tensor.matmul(ps[:], identF[:], oacc[t][:],
                                 start=True, stop=False)
                for j in range(jlo, jhi + 1):
                    nc.tensor.matmul(ps[:], ctxn[j][:, xs], wo(j)[:, os_],
                                     start=False, stop=(j == jhi))
                nc.vector.tensor_copy(oacc[t][:], ps[:])

            # (pair, group) -> (tile, jlo, jhi): jhi <= pair-2 before g4
            # (norm of pair-1 lands at g4), <= pair-1 from g5 on; in pairs 3-5
            # the qp psum bank is busy with qproj between g3 and g11.
            OUT_SCHED = {
                (3, 1): (0, 0, 1), (3, 11): (1, 0, 2),
                (4, 1): (2, 0, 2), (4, 11): (3, 0, 3),
                (5, 1): (4, 0, 3), (5, 11): (5, 0, 4),
                (6, 5): (6, 0, 5), (6, 7): (7, 0, 5),
                (6, 9): (0, 2, 5), (6, 11): (1, 3, 5),
                (7, 5): (2, 3, 6), (7, 6): (3, 4, 6),
                (7, 7): (4, 4, 6), (7, 8): (5, 5, 6),
            }
            TAIL_JLO = {0: 6, 1: 6, 2: 7, 3: 7, 4: 7, 5: 7, 6: 6, 7: 6}

            # Flat software pipeline: scores/exp at step s, ctx lags 2 steps so
            # a pair's last ctx matmuls never head-of-line-block the next
            # pair's scores (the ACT stream stays dense at pair boundaries).
            LAG = 2
            cxs_ = {}
            pgs = {}
            qp_ps = [None]

            def emit_ctx(i, g):
                if g == 0:
                    cxs_[i] = (cxp.tile([65, W], F32, tag="cxA", name="cxA"),
                               cxp.tile([65, W], F32, tag="cxB", name="cxB"))
                cxA, cxB = cxs_[i]
                st, sp = (g == 0), (g == NY - 1)
                nc.tensor.matmul(cxA[:, :], v65[g][:], pgs[(i, g)][:, 0:W],
                                 start=st, stop=sp)
                nc.tensor.matmul(cxB[:, :], v65[g][:], pgs[(i, g)][:, W:2 * W],
                                 start=st, stop=sp)
                del pgs[(i, g)]

            def emit_prologue(i):
                cxA, cxB = cxs_[i]
                Zp = nrm.tile([65, 2 * W], F32, tag="Zp", name="Zp")
                zinv = nrm.tile([65, 2 * W], F32, tag="zinv", name="zinv")
                zinvb = nrm.tile([65, 2 * W], BF16, tag="zinvb", name="zinvb")
                nc.vector.tensor_copy(Zp[64:65, 0:W], cxA[64:65, :])
                nc.vector.tensor_copy(Zp[64:65, W:2 * W], cxB[64:65, :])
                # evacuate ctx first: releases the cx PSUM banks for the next
                # pair's ctx accumulation before the slow reciprocal runs
                cxsA = nrm.tile([64, W], BF16, tag="cxsA", name="cxsA")
                cxsB = nrm.tile([64, W], BF16, tag="cxsB", name="cxsB")
                nc.vector.tensor_copy(cxsA[:], cxA[0:64, :])
                nc.vector.tensor_copy(cxsB[:], cxB[0:64, :])
                # custom DVE op requires base partition 0: run over all 65
                # rows (row 64 holds Z_A|Z_B, the rest are don't-care lanes)
                nc.vector.reciprocal_approx_fast(zinv[:], Zp[:])
                nc.vector.tensor_copy(zinvb[64:65, :], zinv[64:65, :])
                del cxs_[i]

                def norm_tail():
                    zbA = zbp.tile([64, W], F32, tag="zb", name="zbA")
                    nc.tensor.matmul(zbA[:], onesEb[64:65, :], zinvb[64:65, 0:W],
                                     start=True, stop=True, tile_position=(64, 0))
                    nc.vector.tensor_mul(ctxn[i][0:64, :], cxsA[:], zbA[:])
                    zbB = zbp.tile([64, W], F32, tag="zb", name="zbB")
                    nc.tensor.matmul(zbB[:], onesEb[64:65, :],
                                     zinvb[64:65, W:2 * W],
                                     start=True, stop=True, tile_position=(64, 0))
                    cbt = nrm.tile([64, W], BF16, tag="cbt", name="cbt")
                    nc.vector.tensor_mul(cbt[:], cxsB[:], zbB[:])
                    nc.sync.dma_start(ctxn[i][64:128, :], cbt[:])
                return norm_tail

            for s in range(NPAIR * NY + LAG):
                i, g = divmod(s, NY)
                if s < NPAIR * NY:
                    # scores + exp for (i, g)
                    ys = slice(P * g, P * (g + 1))
                    scS = scp.tile([P, 2 * W], F32, tag="sc", name="sc")
                    nc.tensor.matmul(scS[:, 0:W], k3T2[0:64, ys],
                                     qpT[i][0:64, :], start=True, stop=True,
                                     tile_position=(0, 0))
                    nc.tensor.matmul(scS[:, W:2 * W], k3T2[64:128, ys],
                                     qpT[i][64:128, :], start=True, stop=True,
                                     tile_position=(64, 0))
                    pt_ = ptp.tile([P, 2 * W], BF16, tag="pt", name="pt")
                    nc.scalar.activation(pt_[:], scS[:], EXP)
                    pgs[(i, g)] = pt_
                if s >= LAG:
                    ci, cg = divmod(s - LAG, NY)
                    emit_ctx(ci, cg)
                    if cg == NY - 1:
                        pending_prologue[0] = ci
                if s < NPAIR * NY:
                    if g == 2 and pending_prologue[0] is not None:
                        pending_norm[0] = emit_prologue(pending_prologue[0])
                        pending_prologue[0] = None
                    if g == 5 and pending_norm[0] is not None:
                        pending_norm[0]()
                        pending_norm[0] = None
                    if i == 0 and g == 3:
                        # de-prioritized: the kv c1/c2 DMAs land late; the
                        # scheduler must prefer ready scores over these MMs
                        with tc.high_priority(offset=-100000):
                            do_chunk(1)
                    if i == 0 and g == 6:
                        with tc.high_priority(offset=-100000):
                            do_chunk(2)
                    # q projection for pair i+2, one f-tile per group
                    m = i + 2
                    if m < NPAIR and 3 <= g <= 10:
                        f = g - 3
                        if f == 0:
                            qp_ps[0] = qpp.tile([P, CH], F32, tag="qp", name="qp2")
                        nc.tensor.matmul(qp_ps[0][:], wqm(f, m),
                                         qt(f), start=(f == 0), stop=(f == NF - 1))
                    if m < NPAIR and g == 11:
                        nc.vector.tensor_copy(qpT[m][:], qp_ps[0][:])
            # pair 7 epilogue (must run inside this scope: zbp/nrm pools)
            emit_prologue(pending_prologue[0])()

        # ---------------- output projection tail
        with tc.tile_pool(name="opps", bufs=4, space="PSUM") as opp, \
             tc.tile_pool(name="osb", bufs=4) as osb:
            for t in range(8):
                x, o = t // 2, t % 2
                xs = slice(P * x, P * (x + 1))
                os_ = slice(W * o, W * (o + 1))
                ps = opp.tile([P, W], F32, tag="op", name="op")
                for j in range(NPAIR):
                    nc.tensor.matmul(ps[:], ctxn[j][:, xs], wo(j)[:, os_],
                                     start=(j == 0), stop=(j == NPAIR - 1))
                ot = osb.tile([P, W], BF16, tag="os", name="os")
                nc.scalar.copy(ot[:], ps[:])
                nc.sync.dma_start(OUT.ap()[xs, os_], ot[:])

    nc.compile()
    return nc


def _get_nc():
    if "nc" not in _CACHE:
        _CACHE["nc"] = _build()
    return _CACHE["nc"]


def kernel(q, kv, Wq, Wkv, Wo, w=None, _trace=False):
    import ml_dtypes
    from concourse import bass_utils

    BF = ml_dtypes.bfloat16
    F8 = ml_dtypes.float8_e4m3

    q = np.asarray(q, np.float32).reshape(L, DM)
    kv = np.asarray(kv, np.float32).reshape(L, DM)
    Wq = np.asarray(Wq, np.float32)
    Wkv = np.asarray(Wkv, np.float32)
    Wo = np.asarray(Wo, np.float32)

    def tile_rows(a, dt):
        # [DM, C] -> [128, NF*C] with column block f = rows 128f:128(f+1)
        c = a.shape[1]
        out = np.empty((P, a.shape[0] // P * c), dt)
        for f in range(a.shape[0] // P):
            out[:, f * c:(f + 1) * c] = a[P * f:P * (f + 1), :]
        return np.ascontiguousarray(out)

    qT = q.T.astype(np.float32)                      # [DM, L]
    kvT = kv.T.astype(np.float32)                    # [DM, L]
    WQs = (Wq / np.sqrt(DH)).astype(np.float32)      # fold 1/sqrt(d_head)
    WVKf = np.concatenate([Wkv[:, DH:], Wkv[:, :DH]], axis=1)  # [Wv | Wk]

    # wq layout: [128, 2*NF*512]: block (h, f) = WQs[128f:128(f+1), 512h:512(h+1)]
    wq_t = np.empty((P, 2 * NF * (DM // 2)), BF)
    for h in range(2):
        for f in range(NF):
            wq_t[:, 4096 * h + 512 * f: 4096 * h + 512 * (f + 1)] = \
                WQs[P * f:P * (f + 1), 512 * h:512 * (h + 1)].astype(BF)
    wq_t = np.ascontiguousarray(wq_t)
    wvk_t = tile_rows(WVKf, BF)
    wo_t = tile_rows(Wo, BF)

    in_maps = []
    for c in range(NCORES):
        kvt_c = np.zeros((DM, YW), np.float32)
        lo = (c - 1) * CH
        hi = (c + 2) * CH
        src_lo, src_hi = max(lo, 0), min(hi, L)
        dst_lo = src_lo - lo
        kvt_c[:, dst_lo:dst_lo + (src_hi - src_lo)] = kvT[:, src_lo:src_hi]
        # kv layout: [128, 3*NF*512]: block (n, f) = chunk n, rows 128f:
        kv_t = np.empty((P, 3 * NF * W), BF)
        for n in range(3):
            for f in range(NF):
                kv_t[:, (n * NF + f) * W:(n * NF + f + 1) * W] = \
                    kvt_c[P * f:P * (f + 1), W * n:W * (n + 1)].astype(BF)
        in_maps.append({
            "QT": tile_rows(np.ascontiguousarray(qT[:, c * CH:(c + 1) * CH]), BF),
            "KVT": np.ascontiguousarray(kv_t),
            "WQ": wq_t,
            "WVK": wvk_t,
            "WO": wo_t,
        })

    nc = _get_nc()
    res = bass_utils.run_bass_kernel_spmd(
        nc, in_maps, core_ids=list(range(NCORES)), trace=_trace)
    if _trace:
        _CACHE["last_result"] = res

    out = np.concatenate([np.asarray(r["OUT"]).astype(np.float32)
                          for r in res.results], axis=0)
    return out.reshape(B, L, DM)
